# revision 5
# baseline (speedup 1.0000x reference)
"""Bass/Trainium2 kernel for nn_DecorrelationGradient.

Reference computation (KAPPA = 0.5):
    out = (1-k)*(gram - diag_ms) + k*(diag_ms - 1)
        = 0.5 * (X^T X / N) - 0.5          (diag terms cancel algebraically)

with X = x.reshape(N, d), N = 8*2048 = 16384, d = 768.

Strategy (data-parallel over the sample axis, 8 cores):
  - core c gets x[c] : [2048, 768] f32
  - per-core pipeline: HWDGE f32 loads (both rings, 16 single-tile DMAs:
    measured at the per-core HBM roofline ~358 GB/s) -> casts -> PE
    accumulates the upper-triangle blocks of the partial Gram
    P_c = x_c^T x_c in PSUM (fp32)
  - PE dtype schedule: k-tiles 0-1 run as bf16 single-k matmuls (so the
    PE can start as soon as the FIRST tile lands); k-tiles 2-15 run as 7
    fp8e4m3 DoubleRow super-tiles (256-sample contraction, ~2x PE rate).
    fp8 quantization adds only ~2e-4 rel err: the Gram averages 16384
    independent products, so per-product noise shrinks ~sqrt(N).
  - casts alternate engines by tile parity (even k -> ACT activation
    copy, odd k -> DVE tensor_copy) so the last tile's cast never queues
    behind the second-to-last tile's cast on the same engine. k15 (the
    critical last tile) is cast in two pieces on BOTH engines in
    parallel: [0:512] on DVE, [512:768] on ACT.
  - junk matmuls on a zeroed tile bridge the PE from t=0 to the first
    real tile so the HAM clock-gate un-throttles before real work
  - the final super-tile's chunks run in completion order (big blocks
    first). Each chunk's scaled+biased PSUM->SBUF fp16 copy
    (t = P_c * (0.5/N) - 0.5/8) fires immediately after that chunk's
    stop-matmul, alternating ACT/DVE with balanced column counts, so
    copies overlap the remaining finale matmuls. Each row-block's store
    fires as soon as its chunks are copied, alternating the two HWDGE
    rings; the last store is the tiny 32KB block 5, which minimizes the
    final HBM-write completion latency that gates the NEFF exit barrier.
  - each core outputs its scaled partial packed triangle [128, 2688]
    fp16; the host gather sums the 8 partials in fp32 (the affine above
    makes the sum equal 0.5*G/N - 0.5) and unpacks the symmetric matrix.
"""

import numpy as np

import concourse.bacc as bacc
import concourse.bass as bass  # noqa: F401
import concourse.tile as tile
from concourse import mybir
from concourse.bass_utils import run_bass_kernel_spmd

P = 128
D = 768
NSHARD = 2048          # samples per core
KT = NSHARD // P       # 16 k-tiles
KBF = 2                # leading k-tiles computed in bf16 (early PE start)
NSUPER = (KT - KBF) // 2
NB = D // P            # 6 row/col blocks
NCORES = 8
NTOT = 8 * 2048
SCALE = 0.5 / NTOT     # 2**-15, exact
BIAS = -0.5 / NCORES   # -0.0625, exact in fp16; host sum of 8 -> -0.5

# packed upper-triangle blocks (i, j) with j >= i, row-major in i
TRI_BLOCKS = [(i, j) for i in range(NB) for j in range(i, NB)]
NTRI = len(TRI_BLOCKS)          # 21
TRI_W = NTRI * P                # 2688 packed columns

# packed column range of row-block i
OFFS = []
_o = 0
for _i in range(NB):
    OFFS.append((_o, _o + (NB - _i) * P))
    _o = OFFS[-1][1]


def _split_free(width):
    """Split a moving free-dim into chunks <= 512 (one PSUM bank of fp32)."""
    out = []
    s = 0
    while s < width:
        w = min(512, width - s)
        out.append((s, s + w))
        s += w
    return out


def _build():
    nc = bacc.Bacc(num_devices=NCORES)

    x_sh = nc.dram_tensor(
        "x_shard", [NSHARD, D], mybir.dt.float32, kind="ExternalInput"
    )
    out_sh = nc.dram_tensor(
        "out_shard", [P, TRI_W], mybir.dt.float16, kind="ExternalOutput"
    )

    f32 = mybir.dt.float32
    bf16 = mybir.dt.bfloat16
    f16 = mybir.dt.float16
    f8 = mybir.dt.float8e4  # e4m3

    with tile.TileContext(nc) as tc:
        with (
            tc.tile_pool(name="xp", bufs=KT) as xpool,
            tc.tile_pool(name="bp", bufs=KBF + 1) as bpool,
            tc.tile_pool(name="f8p", bufs=NSUPER) as f8pool,
            tc.tile_pool(name="ps", bufs=1, space="PSUM") as pspool,
            tc.tile_pool(name="acc", bufs=1) as accpool,
        ):
            # HAM warmup tile: first thing in program order so the junk
            # matmuls fill the PE pipe while the first x tiles stream in
            warm = bpool.tile([P, 512], bf16, tag="warm", name="warm")
            nc.gpsimd.memset(warm[:], 0.0)

            # load pipeline: 16 single-tile HWDGE f32 DMAs alternating the
            # two physical rings (SP / ACT) — measured at the per-core HBM
            # roofline; single-tile granularity keeps first-arrival
            # latency low so the PE never starves.
            # Casts run on DVE (which issues no DMAs, so its FIFO can
            # freely block on tile sems) EXCEPT the two last-arriving
            # tiles: k14's cast goes to ACT and is emitted after the DMA
            # loop (so it sits behind all of ACT's dma issues in FIFO
            # order), and k15 is cast in two pieces on BOTH engines in
            # parallel — so the finale never queues behind another cast.
            xt_bf = []
            f8t = []
            for s in range(NSUPER):
                f8t.append(
                    f8pool.tile([P, 2, D], f8, tag="f8", name=f"f8_{s}")
                )

            stages = []
            for k in range(KT):
                stage = xpool.tile([P, D], f32, tag="xs", name=f"xs{k}")
                dma_eng = nc.sync if k % 2 == 0 else nc.scalar
                dma_eng.dma_start(out=stage[:], in_=x_sh[k * P : (k + 1) * P, :])
                stages.append(stage)
                if k < KBF:
                    xtile = bpool.tile([P, D], bf16, tag="xb", name=f"xb{k}")
                    nc.vector.tensor_copy(out=xtile[:], in_=stage[:])
                    xt_bf.append(xtile)
                elif k < KT - 2:
                    s, t = divmod(k - KBF, 2)
                    nc.vector.tensor_copy(out=f8t[s][:, t, :], in_=stage[:])
            # k14 on ACT (after all dma issues), k15 split DVE/ACT
            nc.scalar.activation(
                out=f8t[NSUPER - 1][:, 0, :],
                in_=stages[KT - 2][:],
                func=mybir.ActivationFunctionType.Copy,
            )
            nc.vector.tensor_copy(
                out=f8t[NSUPER - 1][:, 1, 0:512], in_=stages[KT - 1][:, 0:512]
            )
            nc.scalar.activation(
                out=f8t[NSUPER - 1][:, 1, 512:D],
                in_=stages[KT - 1][:, 512:D],
                func=mybir.ActivationFunctionType.Copy,
            )

            tri = accpool.tile([P, TRI_W], f16)  # packed scaled triangle

            # psum accumulators, one per row-block; exactly 8 PSUM banks.
            # row-block i covers G[i-block, j-blocks j>=i] = cols 128*i..768
            pss = []
            for i in range(NB):
                pss.append(
                    pspool.tile([P, D - P * i], f32, tag=f"ps{i}", name=f"ps{i}")
                )

            # HAM warmup: junk matmuls on the zeroed tile keep the PE busy
            # (~4us cold) until the first real tile lands, so the HAM
            # activity window that un-throttles the PE clock fires early.
            # Junk goes to pss[0]; the real k=0 matmul has start=True
            # which resets it.
            for w in range(9):
                nc.tensor.matmul(
                    pss[0][:, 0:512],
                    lhsT=warm[:, 0:P],
                    rhs=warm[:],
                    start=True,
                    stop=True,
                )

            # per-tile matmul chunk list, ordered so consecutive matmuls
            # use different stationary weights (the 2nd chunk of i=0/i=1
            # is deferred) - lets LDWEIGHTS overlap the running matmul
            chunks = []  # (i, s0, s1)
            deferred = []
            for i in range(NB):
                sp = _split_free(D - P * i)
                chunks.append((i, sp[0][0], sp[0][1]))
                for s0, s1 in sp[1:]:
                    deferred.append((i, s0, s1))
            chunks[2:2] = deferred  # order: i0a, i1a, i0b, i1b, i2..i5

            # final super-tile: completion order (all chunks of block i
            # adjacent, big blocks first) so each chunk's copy-out and
            # each row-block's store fire as early as possible and
            # overlap the remaining matmuls
            last_chunks = []
            for i in range(NB):
                for s0, s1 in _split_free(D - P * i):
                    last_chunks.append((i, s0, s1))

            # finale copy-out plan: 7 copy ops balanced across ACT/DVE
            # (ACT ~(N+352)/1.2 ns, DVE ~(N+170)/0.96 ns), each fired as
            # soon as its source chunk's stop-matmul lands, queues
            # monotone in matmul completion order so the strict-FIFO
            # engines never stall:
            #   ACT: b0a (512), b1 (640), b4 (256)     ~2.0 us
            #   DVE: b0b (256), b2 (512), b3 (384), b5 (128)  ~1.9 us
            # copy key: (block, rel col range) -> engine
            copy_plan = {
                (0, 0, 512): "act",
                (0, 512, 768): "dve",
                (1, 0, 640): "act",
                (2, 0, 512): "dve",
                (3, 0, 384): "dve",
                (4, 0, 256): "act",
                (5, 0, 128): "dve",
            }

            def copy_piece(i, s0, s1):
                o0, _ = OFFS[i]
                if copy_plan[(i, s0, s1)] == "act":
                    nc.scalar.activation(
                        out=tri[:, o0 + s0 : o0 + s1],
                        in_=pss[i][:, s0:s1],
                        func=mybir.ActivationFunctionType.Copy,
                        scale=SCALE,
                        bias=BIAS,
                    )
                else:
                    nc.vector.tensor_scalar(
                        out=tri[:, o0 + s0 : o0 + s1],
                        in0=pss[i][:, s0:s1],
                        scalar1=SCALE,
                        scalar2=BIAS,
                        op0=mybir.AluOpType.mult,
                        op1=mybir.AluOpType.add,
                    )

            # bf16 prologue: k = 0, 1 as single-k matmuls (PE starts on
            # the very first tile)
            for k in range(KBF):
                for i, s0, s1 in chunks:
                    c0 = P * i
                    nc.tensor.matmul(
                        pss[i][:, s0:s1],
                        lhsT=xt_bf[k][:, c0 : c0 + P],
                        rhs=xt_bf[k][:, c0 + s0 : c0 + s1],
                        start=(k == 0),
                        stop=False,
                    )

            # fp8 DoubleRow main loop: super-tile s = (k 2s+2, 2s+3),
            # 256-sample contraction per matmul at ~2x PE rate. The final
            # super runs its chunks in completion order; each finished
            # chunk's copy-out fires immediately, and each finished
            # row-block's store follows, all overlapping the remaining
            # matmuls.
            # copy pieces fired after the chunk that completes them, and
            # store groups fired after the copy that completes them:
            #   {0} sync, {1,2} sync, {3,4} scalar, {5} sync
            # (the tiny trailing {5} minimizes the last HBM-write
            # completion latency that gates the NEFF exit barrier)
            copy_after = {
                (0, 0, 512): [(0, 0, 512)],
                (0, 512, 768): [(0, 512, 768)],
                (1, 512, 640): [(1, 0, 640)],
                (2, 0, 512): [(2, 0, 512)],
                (3, 0, 384): [(3, 0, 384)],
                (4, 0, 256): [(4, 0, 256)],
                (5, 0, 128): [(5, 0, 128)],
            }
            store_after = {
                (0, 512, 768): (nc.sync, OFFS[0][0], OFFS[0][1]),
                (2, 0, 512): (nc.sync, OFFS[1][0], OFFS[2][1]),
                (4, 0, 256): (nc.scalar, OFFS[3][0], OFFS[4][1]),
                (5, 0, 128): (nc.sync, OFFS[5][0], OFFS[5][1]),
            }

            for s in range(NSUPER):
                last = s == NSUPER - 1
                ch = last_chunks if last else chunks
                for i, s0, s1 in ch:
                    c0 = P * i
                    nc.tensor.matmul(
                        pss[i][:, s0:s1],
                        lhsT=f8t[s][:, :, c0 : c0 + P],
                        rhs=f8t[s][:, :, c0 + s0 : c0 + s1],
                        start=False,
                        stop=last,
                        perf_mode=mybir.MatmulPerfMode.DoubleRow,
                    )
                    if last:
                        for piece in copy_after.get((i, s0, s1), []):
                            copy_piece(*piece)
                        if (i, s0, s1) in store_after:
                            eng, o0, o1 = store_after[(i, s0, s1)]
                            eng.dma_start(
                                out=out_sh[:, o0:o1], in_=tri[:, o0:o1]
                            )

    nc.finalize()  # Bacc: run reg-alloc + wait-legalization passes
    return nc


_NC_CACHE = None

# test-harness hooks (harness calls kernel() only; these stay defaults there)
RUN_KWARGS = {}
LAST_RESULTS = None


def _get_nc():
    global _NC_CACHE
    if _NC_CACHE is None:
        _NC_CACHE = _build()
    return _NC_CACHE


def kernel(x: np.ndarray) -> np.ndarray:
    global LAST_RESULTS
    x = np.ascontiguousarray(np.asarray(x, dtype=np.float32))
    assert x.shape == (NCORES, NSHARD, D)

    nc = _get_nc()
    in_maps = [{"x_shard": x[c]} for c in range(NCORES)]
    res = run_bass_kernel_spmd(
        nc, in_maps, core_ids=list(range(NCORES)), **RUN_KWARGS
    )
    LAST_RESULTS = res

    # gather/unshard: sum the per-core scaled partial triangles (fp16) in
    # fp32, then unpack the symmetric matrix
    packed = np.zeros((P, TRI_W), dtype=np.float32)
    for c in range(NCORES):
        packed += res.results[c]["out_shard"].astype(np.float32)
    packed = packed.reshape(P, NTRI, P).transpose(1, 0, 2)  # [21, 128, 128]

    out = np.empty((D, D), dtype=np.float32)
    for b, (i, j) in enumerate(TRI_BLOCKS):
        blk = packed[b]
        out[P * i : P * (i + 1), P * j : P * (j + 1)] = blk
        if j != i:
            out[P * j : P * (j + 1), P * i : P * (i + 1)] = blk.T
    return out
